# revision 9
# baseline (speedup 1.0000x reference)
"""Trainium2 Bass kernel for nn_Dense_BinaryLayer (binary-weight dense layer).

out = x @ Wb + b, where Wb = binarize(W) in {-1, +1}.

Strategy: data-parallel over the 8 NeuronCores — each core handles 2048 rows
of x and the full (replicated) W and b; no collectives.  Host-side prep is
pure data movement: each core's x slice is permuted into per-row-tile
k-major blocks ([it, p, kt, r] layout, so every DMA is one contiguous
256 KiB DRAM read with 2 KiB per-partition segments), and both x and W are
passed as the high 16 bits of each f32 (byte-slice view = bf16 truncation,
no arithmetic; rel err ~3e-3 vs the 2e-2 gate, verified).

Compute path (fp8 DoubleRow, 2x PE throughput over bf16):
  - Scalar engine binarizes W chunks as they land with a single Sign
    activation (sign(W - 2^-24) in {-1,+1}; W's value grid contains no
    exact 2^-24, so no zeros), emitting wb directly in fp8e4.
  - DVE splits each bf16 x tile exactly into two fp8e4 tiles:
    a = fp8(x), bq = fp8(x - a).  e4m3 holds 4 mantissa bits, so a+bq
    reconstructs all 8 bf16 mantissa bits exactly (Dekker split); only
    |x| < 0.25 sees sub-subnormal rounding, which is negligible.
  - TensorE runs 2 accumulating fp8 DoubleRow passes (a@Wb + bq@Wb) per
    PSUM tile, contracting 2 k-subtiles per instruction.
  - GpSimd adds the broadcast bias on PSUM eviction; per-row-tile stores
    rotate across the three DMA rings.
"""
import sys

sys.path.insert(0, "/opt/trn_rl_repo")

import numpy as np

N_TOTAL = 16384
D_IN = 1024
D_OUT = 1024
N_CORES = 8
ROWS = N_TOTAL // N_CORES      # 2048 rows per core
P = 128
K_TILES = D_IN // P            # 8
I_TILES = ROWS // P            # 16
BIN_THRESH = 2.0 ** -24

_cached = {}


def _build():
    import concourse.tile as tile
    from concourse import bacc, mybir

    f32 = mybir.dt.float32
    bf16 = mybir.dt.bfloat16
    fp8 = mybir.dt.float8e4
    TS = mybir.AluOpType
    DR = mybir.MatmulPerfMode.DoubleRow
    AF = mybir.ActivationFunctionType

    nc = bacc.Bacc()
    xt_d = nc.declare_dram_parameter(
        "xT", [I_TILES * P, K_TILES * P], bf16, isOutput=False)
    w_d = nc.declare_dram_parameter("W", [D_IN, D_OUT], bf16, isOutput=False)
    b_d = nc.declare_dram_parameter("b", [D_OUT], f32, isOutput=False)
    o_d = nc.declare_dram_parameter("out", [ROWS, D_OUT], f32, isOutput=True)

    with tile.TileContext(nc) as tc:
        with (
            tc.tile_pool(name="const", bufs=1) as const,
            tc.tile_pool(name="wpool", bufs=1) as wpool,
            tc.tile_pool(name="xts", bufs=I_TILES) as xts,
            tc.tile_pool(name="x8s", bufs=I_TILES) as x8s,
            tc.tile_pool(name="outp", bufs=8) as outp,
            tc.tile_pool(name="pso", bufs=4, space="PSUM") as pso,
        ):
            xt_ap = xt_d[:].rearrange("(it p) (kt r) -> it p kt r", p=P, kt=K_TILES)
            w_ap = w_d[:].rearrange("(kt p) j -> p kt j", p=P)

            w_raw = wpool.tile([P, K_TILES, D_OUT], bf16, tag="wraw")
            x_tiles = []
            for it in range(I_TILES):
                x_tiles.append(
                    xts.tile([P, K_TILES, P], bf16, tag="x", name=f"xt_{it}"))

            # interleave first x tiles and W chunks across the two HWDGE
            # rings in consumption order; bulk x via SWDGE
            nc.sync.dma_start(x_tiles[0][:], xt_ap[0])
            nc.scalar.dma_start(w_raw[:, 0, :], w_ap[:, 0, :])
            nc.sync.dma_start(x_tiles[1][:], xt_ap[1])
            nc.scalar.dma_start(w_raw[:, 1, :], w_ap[:, 1, :])
            nc.sync.dma_start(x_tiles[2][:], xt_ap[2])
            nc.scalar.dma_start(w_raw[:, 2, :], w_ap[:, 2, :])
            nc.sync.dma_start(x_tiles[3][:], xt_ap[3])
            for kt in range(3, K_TILES):
                nc.scalar.dma_start(w_raw[:, kt, :], w_ap[:, kt, :])

            # bias broadcast to all partitions
            bb = const.tile([P, D_OUT], f32, tag="bb")
            nc.sync.dma_start(bb[:], b_d[:].unsqueeze(0).partition_broadcast(P))

            # per-partition scalar constant for the Sign-binarize bias
            thr = const.tile([P, 1], f32, tag="thr")
            nc.gpsimd.memset(thr[:], -BIN_THRESH)

            for it in range(4, I_TILES):
                nc.gpsimd.dma_start(x_tiles[it][:], xt_ap[it])

            # binarize on the Scalar engine as chunks land:
            # wb = sign(W - 2^-24) in {-1, +1}, emitted directly as fp8e4
            wb = wpool.tile([P, K_TILES, D_OUT], fp8, tag="wb")
            for kt in range(K_TILES):
                nc.scalar.activation(
                    wb[:, kt, :], w_raw[:, kt, :], AF.Sign, bias=thr[:],
                )

            # DVE: exact Dekker split of each x tile into fp8 hi+lo
            a_tiles, b_tiles = [], []
            for it in range(I_TILES):
                a8 = x8s.tile([P, K_TILES, P], fp8, tag="a8", name=f"a8_{it}")
                b8 = x8s.tile([P, K_TILES, P], fp8, tag="b8", name=f"b8_{it}")
                a_tiles.append(a8)
                b_tiles.append(b8)

            for it in range(I_TILES):
                src = x_tiles[it]
                a8, b8 = a_tiles[it], b_tiles[it]
                nc.scalar.copy(a8[:], src[:])
                nc.vector.tensor_tensor(
                    out=b8[:], in0=src[:], in1=a8[:], op=TS.subtract,
                )
                ps_o = pso.tile([P, D_OUT], f32, tag="pso", name=f"pso_{it}")
                for p_i, s8 in ((0, a8), (1, b8)):
                    for ktp in range(0, K_TILES, 2):
                        first = p_i == 0 and ktp == 0
                        last = p_i == 1 and ktp == K_TILES - 2
                        nc.tensor.matmul(
                            ps_o[:, 0:512],
                            s8[:, ktp:ktp + 2, :],
                            wb[:, ktp:ktp + 2, 0:512],
                            start=first, stop=last, perf_mode=DR,
                        )
                        nc.tensor.matmul(
                            ps_o[:, 512:1024],
                            s8[:, ktp:ktp + 2, :],
                            wb[:, ktp:ktp + 2, 512:1024],
                            start=first, stop=last, perf_mode=DR,
                        )
                out_sb = outp.tile([P, D_OUT], f32, tag="out", name=f"out_{it}")
                nc.vector.tensor_tensor(
                    out=out_sb[:], in0=ps_o[:], in1=bb[:], op=TS.add,
                )
                ring = (nc.sync, nc.scalar, nc.gpsimd)[it % 3]
                ring.dma_start(o_d[it * P:(it + 1) * P, :], out_sb[:])

    nc.compile()
    nc.finalize()
    return nc


def _hi16(a):
    """bf16 truncation of a C-contiguous f32 array as a byte-slice view."""
    import ml_dtypes

    u = a.view(np.uint16).reshape(*a.shape, 2)[..., 1]
    return np.ascontiguousarray(u).view(ml_dtypes.bfloat16)


def make_in_maps(x, W, b):
    x = np.ascontiguousarray(np.asarray(x, dtype=np.float32))
    W = np.ascontiguousarray(np.asarray(W, dtype=np.float32))
    b = np.ascontiguousarray(np.asarray(b, dtype=np.float32))
    W16 = _hi16(W)
    maps = []
    for c in range(N_CORES):
        xc = x[c * ROWS:(c + 1) * ROWS]
        # [it, r, kt, p] -> [it, p, kt, r]: row-tile blocks, k-major inside
        blk = np.ascontiguousarray(
            xc.reshape(I_TILES, P, K_TILES, P).transpose(0, 3, 2, 1))
        maps.append({
            "xT": _hi16(blk).reshape(I_TILES * P, K_TILES * P),
            "W": W16,
            "b": b,
        })
    return maps


def kernel(x, W, b):
    from concourse.bass_utils import run_bass_kernel_spmd

    if "nc" not in _cached:
        _cached["nc"] = _build()
    nc = _cached["nc"]

    in_maps = make_in_maps(x, W, b)
    res = run_bass_kernel_spmd(nc, in_maps, list(range(N_CORES)))
    out = np.concatenate([res.results[c]["out"] for c in range(N_CORES)], axis=0)
    return out.astype(np.float32, copy=False)


# revision 10
# speedup vs baseline: 1.1965x; 1.1965x over previous
"""Trainium2 Bass kernel for nn_Dense_BinaryLayer (binary-weight dense layer).

out = x @ Wb + b, where Wb = binarize(W) in {-1, +1}.

Strategy: data-parallel over the 8 NeuronCores — each core handles 2048 rows
of x and the full (replicated) W and b; no collectives.  Host-side prep is
pure data movement: each core's x slice is permuted into per-row-tile
k-major blocks ([it, p, kt, r] layout, so every DMA is one contiguous
256 KiB DRAM read with 2 KiB per-partition segments), and both x and W are
passed as the high 16 bits of each f32 (byte-slice view = bf16 truncation,
no arithmetic; rel err ~3e-3 vs the 2e-2 gate, verified).  fp8 DoubleRow
was tried and measured: per-instruction time on HW equals bf16's, so the
exact 2-pass fp8 split is a wash — single-pass bf16 is the PE floor
(~57.5us busy).

Schedule (the PE stream executes in order, round-robin over 4 PSUM tiles,
so early tiles must land in consumption order):
  - W streams as 8 contiguous per-k-tile chunks spread over BOTH HWDGE
    rings so every wb k-tile is binarized by ~12us; DVE binarizes each
    chunk as it lands (bf16 2x DVE rate).
  - x row-tiles 0/1 go first on the SP HWDGE ring; the rest stream via
    SWDGE (x14/15 on the Act ring to balance), each with a dedicated SBUF
    buffer (no recycling stalls).
  - bf16 matmuls (free dim 512) accumulate in PSUM over the 8 k-tiles;
    DVE adds the broadcast bias on eviction; per-row-tile stores rotate
    across the three DMA rings.
"""
import sys

sys.path.insert(0, "/opt/trn_rl_repo")

import numpy as np

N_TOTAL = 16384
D_IN = 1024
D_OUT = 1024
N_CORES = 8
ROWS = N_TOTAL // N_CORES      # 2048 rows per core
P = 128
K_TILES = D_IN // P            # 8
I_TILES = ROWS // P            # 16
BIN_THRESH = 2.0 ** -24

_cached = {}


def _build():
    import concourse.tile as tile
    from concourse import bacc, mybir

    f32 = mybir.dt.float32
    bf16 = mybir.dt.bfloat16
    TS = mybir.AluOpType

    nc = bacc.Bacc()
    xt_d = nc.declare_dram_parameter(
        "xT", [I_TILES * P, K_TILES * P], bf16, isOutput=False)
    w_d = nc.declare_dram_parameter("W", [D_IN, D_OUT], bf16, isOutput=False)
    b_d = nc.declare_dram_parameter("b", [D_OUT], f32, isOutput=False)
    o_d = nc.declare_dram_parameter("out", [ROWS, D_OUT], f32, isOutput=True)

    with tile.TileContext(nc) as tc:
        with (
            tc.tile_pool(name="const", bufs=1) as const,
            tc.tile_pool(name="wpool", bufs=1) as wpool,
            tc.tile_pool(name="xts", bufs=I_TILES) as xts,
            tc.tile_pool(name="outp", bufs=8) as outp,
            tc.tile_pool(name="pso", bufs=4, space="PSUM") as pso,
        ):
            xt_ap = xt_d[:].rearrange("(it p) (kt r) -> it p kt r", p=P, kt=K_TILES)
            w_ap = w_d[:].rearrange("(kt p) j -> p kt j", p=P)

            w_raw = wpool.tile([P, K_TILES, D_OUT], bf16, tag="wraw")
            x_tiles = []
            for it in range(I_TILES):
                x_tiles.append(
                    xts.tile([P, K_TILES, P], bf16, tag="x", name=f"xt_{it}"))

            # earliest-needed data first on each ring, W spread over both
            # HWDGE rings so all 8 chunks land by ~12us
            nc.sync.dma_start(x_tiles[0][:], xt_ap[0])
            nc.scalar.dma_start(w_raw[:, 0, :], w_ap[:, 0, :])
            nc.sync.dma_start(x_tiles[1][:], xt_ap[1])
            nc.scalar.dma_start(w_raw[:, 1, :], w_ap[:, 1, :])
            nc.sync.dma_start(w_raw[:, 2, :], w_ap[:, 2, :])
            nc.scalar.dma_start(w_raw[:, 3, :], w_ap[:, 3, :])
            nc.sync.dma_start(w_raw[:, 4, :], w_ap[:, 4, :])
            nc.scalar.dma_start(w_raw[:, 5, :], w_ap[:, 5, :])
            nc.sync.dma_start(w_raw[:, 6, :], w_ap[:, 6, :])
            nc.scalar.dma_start(w_raw[:, 7, :], w_ap[:, 7, :])

            # bias broadcast to all partitions
            bb = const.tile([P, D_OUT], f32, tag="bb")
            nc.sync.dma_start(bb[:], b_d[:].unsqueeze(0).partition_broadcast(P))

            # bulk x via SWDGE (in consumption order); last two on Act ring
            for it in range(2, I_TILES - 2):
                nc.gpsimd.dma_start(x_tiles[it][:], xt_ap[it])
            nc.scalar.dma_start(x_tiles[I_TILES - 2][:], xt_ap[I_TILES - 2])
            nc.scalar.dma_start(x_tiles[I_TILES - 1][:], xt_ap[I_TILES - 1])

            # binarize on DVE per k-tile (bf16 => 2x DVE rate):
            # m = (W > c) in {0,1}, then Wb = 2m-1 in {+-1}
            wb = wpool.tile([P, K_TILES, D_OUT], bf16, tag="wb")
            wm = wpool.tile([P, D_OUT], bf16, tag="wm")
            for kt in range(K_TILES):
                nc.vector.tensor_scalar(
                    wm[:], w_raw[:, kt, :], BIN_THRESH, None, TS.is_gt,
                )
                nc.vector.tensor_scalar(
                    wb[:, kt, :], wm[:], 2.0, 1.0, TS.mult, TS.subtract,
                )

            for it in range(I_TILES):
                src = x_tiles[it]
                ps_o = pso.tile([P, D_OUT], f32, tag="pso", name=f"pso_{it}")
                for kt in range(K_TILES):
                    first = kt == 0
                    last = kt == K_TILES - 1
                    nc.tensor.matmul(
                        ps_o[:, 0:512],
                        src[:, kt, :],
                        wb[:, kt, 0:512],
                        start=first, stop=last,
                    )
                    nc.tensor.matmul(
                        ps_o[:, 512:1024],
                        src[:, kt, :],
                        wb[:, kt, 512:1024],
                        start=first, stop=last,
                    )
                out_sb = outp.tile([P, D_OUT], f32, tag="out", name=f"out_{it}")
                nc.vector.tensor_tensor(
                    out=out_sb[:], in0=ps_o[:], in1=bb[:], op=TS.add,
                )
                ring = (nc.sync, nc.scalar, nc.gpsimd)[it % 3]
                ring.dma_start(o_d[it * P:(it + 1) * P, :], out_sb[:])

    nc.compile()
    nc.finalize()
    return nc


def _hi16(a):
    """bf16 truncation of a C-contiguous f32 array as a byte-slice view."""
    import ml_dtypes

    u = a.view(np.uint16).reshape(*a.shape, 2)[..., 1]
    return np.ascontiguousarray(u).view(ml_dtypes.bfloat16)


def make_in_maps(x, W, b):
    x = np.ascontiguousarray(np.asarray(x, dtype=np.float32))
    W = np.ascontiguousarray(np.asarray(W, dtype=np.float32))
    b = np.ascontiguousarray(np.asarray(b, dtype=np.float32))
    W16 = _hi16(W)
    maps = []
    for c in range(N_CORES):
        xc = x[c * ROWS:(c + 1) * ROWS]
        # [it, r, kt, p] -> [it, p, kt, r]: row-tile blocks, k-major inside
        blk = np.ascontiguousarray(
            xc.reshape(I_TILES, P, K_TILES, P).transpose(0, 3, 2, 1))
        maps.append({
            "xT": _hi16(blk).reshape(I_TILES * P, K_TILES * P),
            "W": W16,
            "b": b,
        })
    return maps


def kernel(x, W, b):
    from concourse.bass_utils import run_bass_kernel_spmd

    if "nc" not in _cached:
        _cached["nc"] = _build()
    nc = _cached["nc"]

    in_maps = make_in_maps(x, W, b)
    res = run_bass_kernel_spmd(nc, in_maps, list(range(N_CORES)))
    out = np.concatenate([res.results[c]["out"] for c in range(N_CORES)], axis=0)
    return out.astype(np.float32, copy=False)
